# revision 46
# baseline (speedup 1.0000x reference)
"""Multi-head attention Trainium2 kernel (B=4, T=2048, C=1024, H=16, D=64).

Sharding: 8 cores = 4 batches x 2 head-groups (data parallel on B, tensor
parallel on H). Each core computes attention for 1 batch and 8 heads plus the
partial out-projection for its head rows; the host sums the two partials per
batch (the out-proj "all-reduce"); bias is applied on-device by hg=0 cores.

Structure per core:
  - Inputs arrive as two packed [128, 16384] dram tensors (xw: x^T in two
    1024-column waves; wall: wk|wq|wv ctiles + wo blocks) so the load is a
    handful of ~1-2MB fully-contiguous DMAs.
  - Scores S^T[k,q] per head pair: two K=64 matmuls on disjoint PE row
    groups run concurrently. exp on ScalarE (the end-to-end bottleneck)
    into a 64-unit bf16 ring holding exactly two sections (no wrap).
  - PV is deferred one section and col-group paired: V natural [t, 64] per
    head, h0 -> PSUM partitions 0:64, h1 -> 64:128 of one bank (~2x over
    the M=65 ones-column serial form). All 16 windows run at the TOP of the
    next section so the windows -> dtree -> M=1 -> normalize chain finishes
    mid-section and never gates the following section's exp or PV.
  - Softmax denominators: bf16 tree folds each head's 16 ring units to 2
    in-place (forward compaction, safe after the PV windows have read
    them), then one M=1 ones-stationary chain per head into PSUM;
    reciprocal + partition broadcast + scale.
  - Out-proj trails pair-3 sections by one; host sums the partials.
"""

import numpy as np
import ml_dtypes

import concourse.bacc as bacc
import concourse.mybir as mybir
import concourse.tile as tile
from concourse.bass_utils import run_bass_kernel_spmd

B, T, C, H, D = 4, 2048, 1024, 16, 64
HPC = 8          # heads per core
PAIRS = HPC // 2
CT = C // 128    # 8 contraction tiles for projections
TT = T // 128    # 16 t-tiles (also k-tiles of attention)
QC = T // 512    # 4 query chunks
JC = C // 512    # 2 out-proj column chunks
BF16 = mybir.dt.bfloat16
F32 = mybir.dt.float32
EXP = mybir.ActivationFunctionType.Exp
RING = 64        # ring units; sections alternate halves 0:32 / 32:64

_CACHED_NC = None


def _emit(nc, tc, xw_d, wall_d, bias_d, y_d):
    import contextlib
    with contextlib.ExitStack() as ctx:
        persist = ctx.enter_context(tc.tile_pool(name="persist", bufs=1))
        work = ctx.enter_context(tc.tile_pool(name="work", bufs=2))
        spsum = ctx.enter_context(tc.tile_pool(name="spsum", bufs=2, space="PSUM"))
        pvsum = ctx.enter_context(tc.tile_pool(name="pvsum", bufs=2, space="PSUM"))
        ppsum = ctx.enter_context(tc.tile_pool(name="ppsum", bufs=2, space="PSUM"))

        # ---- static loads ----
        xall = persist.tile([128, CT * T], BF16, tag="xall", name="xall")
        wall = persist.tile([128, 16384], BF16, tag="wall", name="wall")

        def xsl(i, c0, c1):
            w = c0 // 1024
            assert c1 <= (w + 1) * 1024
            base = w * 8192 + i * 1024 - w * 1024
            return xall[:, base + c0:base + c1]

        w_sb = {}
        for wi, wname in enumerate(("wk", "wq", "wv")):
            for i in range(CT):
                off = (wi * CT + i) * 512
                w_sb[(wname, i)] = wall[:, off:off + 512]
        wo_sb = [wall[:, 12288 + i * C:12288 + (i + 1) * C] for i in range(4)]
        bias_sb = persist.tile([1, C], F32, tag="bias", name="bias")

        nc.sync.dma_start(out=wall[:, 0:512], in_=wall_d[0:128, 0:512])
        nc.sync.dma_start(out=wall[:, 512:8192], in_=wall_d[0:128, 512:8192])
        nc.scalar.dma_start(out=xall[:, 0:8192], in_=xw_d[0:128, 0:8192])
        nc.gpsimd.dma_start(out=wall[:, 8192:12288],
                            in_=wall_d[0:128, 8192:12288])
        nc.gpsimd.dma_start(out=xall[:, 8192:12288],
                            in_=xw_d[0:128, 8192:12288])
        nc.sync.dma_start(out=xall[:, 12288:16384],
                          in_=xw_d[0:128, 12288:16384])
        nc.scalar.dma_start(out=wall[:, 12288:16384],
                            in_=wall_d[0:128, 12288:16384])
        nc.gpsimd.dma_start(out=bias_sb, in_=bias_d[0:1, :])
        bias_bc = persist.tile([128, C], F32, tag="bias_bc", name="bias_bc")
        nc.gpsimd.partition_broadcast(bias_bc, bias_sb)

        ones = persist.tile([128, 1], BF16, tag="ones", name="ones")
        nc.vector.memset(ones, 1.0)

        # V natural [T, 512] -> vaug tiles [128, 8 heads, 64]
        vaug = [persist.tile([128, HPC, D], BF16, tag=f"vaug{tt}",
                             name=f"vaug{tt}") for tt in range(TT)]

        def v_chunk(tt):
            ps = ppsum.tile([128, 512], F32, tag="proj", name="vps")
            for c in range(CT):
                nc.tensor.matmul(ps, lhsT=xsl(c, tt * 128, (tt + 1) * 128),
                                 rhs=w_sb[("wv", c)], start=(c == 0),
                                 stop=(c == CT - 1))
            nc.vector.tensor_copy(
                out=vaug[tt][:, :, :],
                in_=ps.rearrange("p (h d) -> p h d", h=HPC))

        qt_sb = [persist.tile([128, T], BF16, tag="qt", bufs=2, name=f"qt{p}")
                 for p in range(PAIRS)]
        kt_sb = [persist.tile([128, T], BF16, tag="kt", bufs=2, name=f"kt{p}")
                 for p in range(PAIRS)]

        def project_chunk(p, dst, wname, qc):
            ps = ppsum.tile([128, 512], F32, tag="proj", name="qkps")
            for c in range(CT):
                nc.tensor.matmul(
                    ps,
                    lhsT=w_sb[(wname, c)][:, p * 128:(p + 1) * 128],
                    rhs=xsl(c, qc * 512, (qc + 1) * 512),
                    start=(c == 0), stop=(c == CT - 1))
            nc.vector.tensor_copy(out=dst[:, qc * 512:(qc + 1) * 512], in_=ps)

        # PE warmup on the first wk chunk while the DMA streams in (HAM
        # unthrottle needs ~3.4us of PE busy; also bridges the DMA wait)
        warm = ppsum.tile([128, 512], F32, tag="proj", name="warm")
        for i in range(30):
            nc.tensor.matmul(warm, lhsT=w_sb[("wk", 0)][:, 0:128],
                             rhs=w_sb[("wk", 0)], start=(i == 0), stop=(i == 29))
        # pre-load the exp ACT table set during the DMA window (input is
        # uninitialized scratch; only the side-effect table load matters)
        tl = work.tile([1, 512], F32, tag="ssb", name="tabload")
        nc.scalar.activation(out=tl, in_=tl, func=EXP)

        project_chunk(0, kt_sb[0], "wk", 0)
        project_chunk(0, kt_sb[0], "wk", 1)
        project_chunk(0, qt_sb[0], "wq", 0)

        # ---- attention ----
        exps = persist.tile([128, RING * 512], BF16, tag="expS", name="expS")
        tht_sb = [persist.tile([128, T], BF16, tag=f"tht{p}", name=f"tht{p}")
                  for p in range(PAIRS)]

        fillers = {
            (0, 0): [(0, "wq", 1)],
            (0, 1): [(0, "wq", 2), (1, "wk", 0), (1, "wk", 1)],
            (0, 2): [(0, "wq", 3), (1, "wk", 2), (1, "wk", 3)],
            (0, 3): [(1, "wq", 0), (1, "wq", 1), (1, "wq", 2)],
            (1, 0): [(1, "wq", 3), (2, "wk", 0)],
            (1, 1): [(2, "wk", 1), (2, "wk", 2)],
            (1, 2): [(2, "wk", 3), (2, "wq", 0)],
            (1, 3): [(2, "wq", 1), (2, "wq", 2)],
            (2, 0): [(2, "wq", 3), (3, "wk", 0)],
            (2, 1): [(3, "wk", 1), (3, "wk", 2)],
            (2, 2): [(3, "wk", 3), (3, "wq", 0)],
            (2, 3): [(3, "wq", 1), (3, "wq", 2)],
            (3, 0): [(3, "wq", 3)],
        }

        def out_proj_group(tt):
            ysb = work.tile([128, C], F32, tag="ysb", bufs=2, name="ysb")
            for jc in range(JC):
                jsl = slice(jc * 512, (jc + 1) * 512)
                yps = ppsum.tile([128, 512], F32, tag="proj", name="yps")
                for pp in range(PAIRS):
                    nc.tensor.matmul(
                        yps, lhsT=tht_sb[pp][:, tt * 128:(tt + 1) * 128],
                        rhs=wo_sb[pp][:, jsl],
                        start=(pp == 0), stop=(pp == PAIRS - 1))
                nc.vector.tensor_add(out=ysb[:, jsl], in0=yps,
                                     in1=bias_bc[:, jsl])
            eng = nc.sync if tt % 2 == 0 else nc.gpsimd
            eng.dma_start(out=y_d[tt * 128:(tt + 1) * 128, :], in_=ysb)

        sections = [(p, qc) for p in range(PAIRS) for qc in range(QC)]

        def sec_base(si):
            return (si % 2) * 32

        def unit(si, kt, lh):
            u = sec_base(si) + 2 * kt + lh
            return exps[:, u * 512:(u + 1) * 512]

        def emit_dtree(si):
            # bf16 unit-tree, in-place forward compaction in the ring: after
            # the PV windows have read section si's 32 units, fold each
            # head's 16 units down to 2 (at unit offsets lh + {0, 16}).
            # exp of section si+2 reuses these units and naturally waits via
            # subtile deps on the M=1 chains that consume them.
            b = sec_base(si)
            uv = exps[:, b * 512:(b + 32) * 512].rearrange(
                "p (u q) -> p u q", q=512)
            with nc.allow_low_precision(reason="bf16 exp-sum tree, ~0.3% on D"):
                for lh in range(2):
                    nc.vector.tensor_add(out=uv[:, lh:lh + 29:4, :],
                                         in0=uv[:, lh:lh + 29:4, :],
                                         in1=uv[:, lh + 2:lh + 31:4, :])
                    nc.vector.tensor_add(out=uv[:, lh:lh + 25:8, :],
                                         in0=uv[:, lh:lh + 25:8, :],
                                         in1=uv[:, lh + 4:lh + 29:8, :])
                    nc.vector.tensor_add(out=uv[:, lh:lh + 17:16, :],
                                         in0=uv[:, lh:lh + 17:16, :],
                                         in1=uv[:, lh + 8:lh + 25:16, :])

        def pv_window(si, p, kt, ops):
            for lh in range(2):
                nc.tensor.matmul(
                    ops[64 * lh:64 * (lh + 1), :],
                    lhsT=vaug[kt][:, 2 * p + lh, :],
                    rhs=unit(si, kt, lh),
                    start=(kt == 0), stop=(kt == TT - 1))

        def emit_dchains(si):
            # denominators: one M=1 ones chain per head over the compacted
            # ring units (waits on emit_dtree via subtile deps)
            dps = ppsum.tile([128, 512], F32, tag="proj", name="dps")
            b = sec_base(si)
            for lh in range(2):
                for u in range(2):
                    uu = b + lh + 16 * u
                    nc.tensor.matmul(
                        dps[32 * lh:32 * lh + 1, :],
                        lhsT=ones[:, :],
                        rhs=exps[:, uu * 512:(uu + 1) * 512],
                        start=(u == 0), stop=(u == 1))
            return dps

        def finish_pvd(p, qc, ops, dps):
            qsl = slice(qc * 512, (qc + 1) * 512)
            for lh in range(2):
                ssb = work.tile([1, 512], F32, tag="ssb", name="ssb")
                nc.vector.tensor_copy(out=ssb, in_=dps[32 * lh:32 * lh + 1, :])
                rsb = work.tile([1, 512], F32, tag="rsb", name="rsb")
                nc.vector.reciprocal_approx_fast(out=rsb, in_=ssb)
                rbc = work.tile([64, 512], F32, tag="rbc", name="rbc")
                nc.gpsimd.partition_broadcast(rbc, rsb)
                nc.vector.tensor_mul(
                    out=tht_sb[p][lh * 64:(lh + 1) * 64, qsl],
                    in0=ops[64 * lh:64 * (lh + 1), :], in1=rbc)

        for si, (p, qc) in enumerate(sections):
            qsl = slice(qc * 512, (qc + 1) * 512)
            prev = sections[si - 1] if si > 0 else None
            ops = None
            if prev is not None:
                # PVD of the previous section runs at the TOP of this one:
                # its exps are fully ready, so the whole windows -> dtree ->
                # M=1 -> normalize chain completes by mid-section and never
                # gates the next section's exp (ring WAR) or PV (bank WAR).
                # Section (0,1) is special: its scores go FIRST so ACT never
                # starves while the leftover V-projections and the (0,0)
                # windows weave into the score loop instead.
                ops = pvsum.tile([128, 512], F32, tag="pv", name="ops")
                if si != 1:
                    for kt in range(TT):
                        pv_window(si - 1, prev[0], kt, ops)
                    emit_dtree(si - 1)

            for kt in range(TT):
                if p == 0 and qc == 0 and kt in (8, 12):
                    project_chunk(0, kt_sb[0], "wk", kt // 4)
                ps = spsum.tile([128, 1024], F32, tag="mm", name="sps")
                for lh in range(2):
                    hsl = slice(lh * 64, (lh + 1) * 64)
                    nc.tensor.matmul(
                        ps[:, lh * 512:(lh + 1) * 512],
                        lhsT=kt_sb[p][hsl, kt * 128:(kt + 1) * 128],
                        rhs=qt_sb[p][hsl, qsl],
                        start=True, stop=True)
                u0 = sec_base(si) + 2 * kt
                nc.scalar.activation(
                    out=exps[:, u0 * 512:(u0 + 2) * 512],
                    in_=ps, func=EXP, scale=0.125)
                if p == 0 and qc == 0 and kt >= 6:
                    v_chunk(kt - 6)
                if si == 1:
                    if kt < 6:
                        v_chunk(10 + kt)
                    else:
                        pv_window(0, 0, kt - 6, ops)
                elif prev is not None:
                    if kt == 3:
                        dps = emit_dchains(si - 1)
                    if kt == 5:
                        finish_pvd(prev[0], prev[1], ops, dps)
            if si == 1:
                for k2 in range(10, TT):
                    pv_window(0, 0, k2, ops)
                emit_dtree(0)
                dps = emit_dchains(0)
                finish_pvd(0, 0, ops, dps)
            # out-proj trails pair-3 sections by one: tht3[qc-1] is finalized
            # by finish_pvd at kt==5 of this section
            if p == PAIRS - 1 and qc >= 1:
                for tt in range(4 * (qc - 1), 4 * qc):
                    out_proj_group(tt)
            for fp, wname, fqc in fillers.get((p, qc), []):
                dst = qt_sb[fp] if wname == "wq" else kt_sb[fp]
                project_chunk(fp, dst, wname, fqc)

        # tail: PVD of the last section, then the final out-proj groups
        si = len(sections) - 1
        ops = pvsum.tile([128, 512], F32, tag="pv", name="ops")
        for kt in range(TT):
            pv_window(si, PAIRS - 1, kt, ops)
        emit_dtree(si)
        dps = emit_dchains(si)
        finish_pvd(PAIRS - 1, QC - 1, ops, dps)
        for tt in range(4 * (QC - 1), 4 * QC):
            out_proj_group(tt)


def _build():
    nc = bacc.Bacc("TRN2", target_bir_lowering=False)
    xw_d = nc.dram_tensor("xw", [128, CT * T], BF16, kind="ExternalInput")
    wall_d = nc.dram_tensor("wall", [128, 16384], BF16, kind="ExternalInput")
    bias_d = nc.dram_tensor("bias", [1, C], F32, kind="ExternalInput")
    y_d = nc.dram_tensor("y", [T, C], F32, kind="ExternalOutput")
    with tile.TileContext(nc) as tc:
        _emit(nc, tc, xw_d, wall_d, bias_d, y_d)
    if not nc.is_finalized():
        nc.finalize()
    return nc


def get_nc():
    global _CACHED_NC
    if _CACHED_NC is None:
        _CACHED_NC = _build()
    return _CACHED_NC


def make_in_maps(x, w_qkv, w_out, b_out):
    bf = ml_dtypes.bfloat16
    x = np.asarray(x, dtype=np.float32)
    w_qkv = np.asarray(w_qkv, dtype=np.float32)
    w_out = np.asarray(w_out, dtype=np.float32)
    b_out = np.asarray(b_out, dtype=np.float32)
    in_maps = []
    for core in range(8):
        b, hg = core // 2, core % 2
        cs = slice(hg * 512, (hg + 1) * 512)
        bias = b_out if hg == 0 else np.zeros_like(b_out)
        xT = np.ascontiguousarray(x[b].T).astype(bf)  # [C, T]
        # xw: wave-major pack xw[p, w*8192 + i*1024 + t] = xT[i*128+p, w*1024+t]
        xw = np.ascontiguousarray(
            xT.reshape(CT, 128, 2, 1024).transpose(1, 2, 0, 3)
            .reshape(128, CT * T))
        wq = w_qkv[:, 0 * C:][:, cs].astype(bf).reshape(CT, 128, 512)
        wk = w_qkv[:, 1 * C:][:, cs].astype(bf).reshape(CT, 128, 512)
        wv = w_qkv[:, 2 * C:][:, cs].astype(bf).reshape(CT, 128, 512)
        wo = w_out[cs, :].astype(bf).reshape(4, 128, C)
        wall = np.concatenate(
            [wk.transpose(1, 0, 2).reshape(128, 4096),
             wq.transpose(1, 0, 2).reshape(128, 4096),
             wv.transpose(1, 0, 2).reshape(128, 4096),
             wo.transpose(1, 0, 2).reshape(128, 4096)], axis=1)
        in_maps.append({
            "xw": xw,
            "wall": np.ascontiguousarray(wall),
            "bias": np.ascontiguousarray(bias.reshape(1, C), dtype=np.float32),
        })
    return in_maps


def _ensure_ntff_hook():
    """Register the axon NTFF profile hook if the container's antenv lacks
    axon_hooks (test/profiling use only; never needed for plain kernel())."""
    import sys
    import types
    try:
        from antenv import axon_hooks  # noqa: F401
    except ImportError:
        mod = types.ModuleType("antenv.axon_hooks")
        mod._hook = None

        def set_axon_ntff_profile_hook(hook, _m=mod):
            _m._hook = hook

        def get_axon_ntff_profile_hook(_m=mod):
            return _m._hook

        mod.set_axon_ntff_profile_hook = set_axon_ntff_profile_hook
        mod.get_axon_ntff_profile_hook = get_axon_ntff_profile_hook
        sys.modules["antenv.axon_hooks"] = mod
        import antenv
        antenv.axon_hooks = mod
    import antenv.axon_hooks as ah
    if ah.get_axon_ntff_profile_hook() is None:
        from trn_agent_boot.trn_boot import _ntff_profile_via_ctypes
        ah.set_axon_ntff_profile_hook(
            _ntff_profile_via_ctypes("/opt/axon/libaxon_pjrt.so"))


def kernel(x, w_qkv, w_out, b_out, _trace=False, _trace_kwargs=None):
    nc = get_nc()
    in_maps = make_in_maps(x, w_qkv, w_out, b_out)
    kwargs = {}
    if _trace:
        try:
            _ensure_ntff_hook()
        except Exception as e:
            print(f"NTFF hook setup failed ({e}); running without trace")
        else:
            kwargs.update(trace=True, **(_trace_kwargs or {}))
    res = run_bass_kernel_spmd(nc, in_maps, core_ids=list(range(8)), **kwargs)
    out = np.empty((B, T, C), dtype=np.float32)
    for b in range(B):
        out[b] = res.results[2 * b]["y"] + res.results[2 * b + 1]["y"]
    if _trace:
        return out, res
    return out


# revision 47
# speedup vs baseline: 1.2005x; 1.2005x over previous
"""Multi-head attention Trainium2 kernel (B=4, T=2048, C=1024, H=16, D=64).

Sharding: 8 cores = 4 batches x 2 head-groups (data parallel on B, tensor
parallel on H). Each core computes attention for 1 batch and 8 heads plus the
partial out-projection for its head rows; the host sums the two partials per
batch (the out-proj "all-reduce"); bias is applied on-device by hg=0 cores.

Structure per core:
  - Inputs arrive as two packed [128, 16384] dram tensors (xw: x^T in two
    1024-column waves; wall: wk|wq|wv ctiles + wo blocks) so the load is a
    handful of ~1-2MB fully-contiguous DMAs.
  - Scores S^T[k,q] per head pair: two K=64 matmuls on disjoint PE row
    groups run concurrently. exp on ScalarE (the end-to-end bottleneck)
    into a 64-unit bf16 ring holding exactly two sections (no wrap).
  - PV is deferred one section and col-group paired: V natural [t, 64] per
    head, h0 -> PSUM partitions 0:64, h1 -> 64:128 of one bank (~2x over
    the M=65 ones-column serial form). All 16 windows run at the TOP of the
    next section so the windows -> dtree -> M=1 -> normalize chain finishes
    mid-section and never gates the following section's exp or PV.
  - Softmax denominators: bf16 tree folds each head's 16 ring units to 2
    in-place (forward compaction, safe after the PV windows have read
    them), then one M=1 ones-stationary chain per head into PSUM;
    reciprocal + partition broadcast + scale.
  - Out-proj trails pair-3 sections by one; host sums the partials.
"""

import numpy as np
import ml_dtypes

import concourse.bacc as bacc
import concourse.mybir as mybir
import concourse.tile as tile
from concourse.bass_utils import run_bass_kernel_spmd

B, T, C, H, D = 4, 2048, 1024, 16, 64
HPC = 8          # heads per core
PAIRS = HPC // 2
CT = C // 128    # 8 contraction tiles for projections
TT = T // 128    # 16 t-tiles (also k-tiles of attention)
QC = T // 512    # 4 query chunks
JC = C // 512    # 2 out-proj column chunks
BF16 = mybir.dt.bfloat16
F32 = mybir.dt.float32
EXP = mybir.ActivationFunctionType.Exp
RING = 64        # ring units; sections alternate halves 0:32 / 32:64

_CACHED_NC = None


def _emit(nc, tc, xw_d, wall_d, bias_d, y_d):
    import contextlib
    with contextlib.ExitStack() as ctx:
        persist = ctx.enter_context(tc.tile_pool(name="persist", bufs=1))
        work = ctx.enter_context(tc.tile_pool(name="work", bufs=2))
        spsum = ctx.enter_context(tc.tile_pool(name="spsum", bufs=2, space="PSUM"))
        pvsum = ctx.enter_context(tc.tile_pool(name="pvsum", bufs=2, space="PSUM"))
        ppsum = ctx.enter_context(tc.tile_pool(name="ppsum", bufs=2, space="PSUM"))

        # ---- static loads ----
        xall = persist.tile([128, CT * T], BF16, tag="xall", name="xall")
        wall = persist.tile([128, 16384], BF16, tag="wall", name="wall")

        def xsl(i, c0, c1):
            w = c0 // 1024
            assert c1 <= (w + 1) * 1024
            base = w * 8192 + i * 1024 - w * 1024
            return xall[:, base + c0:base + c1]

        w_sb = {}
        for wi, wname in enumerate(("wk", "wq", "wv")):
            for i in range(CT):
                off = (wi * CT + i) * 512
                w_sb[(wname, i)] = wall[:, off:off + 512]
        wo_sb = [wall[:, 12288 + i * C:12288 + (i + 1) * C] for i in range(4)]
        bias_sb = persist.tile([1, C], F32, tag="bias", name="bias")

        nc.sync.dma_start(out=wall[:, 0:512], in_=wall_d[0:128, 0:512])
        nc.sync.dma_start(out=wall[:, 512:8192], in_=wall_d[0:128, 512:8192])
        nc.scalar.dma_start(out=xall[:, 0:8192], in_=xw_d[0:128, 0:8192])
        nc.gpsimd.dma_start(out=wall[:, 8192:12288],
                            in_=wall_d[0:128, 8192:12288])
        nc.gpsimd.dma_start(out=xall[:, 8192:12288],
                            in_=xw_d[0:128, 8192:12288])
        nc.sync.dma_start(out=xall[:, 12288:16384],
                          in_=xw_d[0:128, 12288:16384])
        nc.scalar.dma_start(out=wall[:, 12288:16384],
                            in_=wall_d[0:128, 12288:16384])
        nc.gpsimd.dma_start(out=bias_sb, in_=bias_d[0:1, :])
        bias_bc = persist.tile([128, C], F32, tag="bias_bc", name="bias_bc")
        nc.gpsimd.partition_broadcast(bias_bc, bias_sb)

        ones = persist.tile([128, 1], BF16, tag="ones", name="ones")
        nc.vector.memset(ones, 1.0)

        # V natural [T, 512] -> vaug tiles [128, 8 heads, 64]
        vaug = [persist.tile([128, HPC, D], BF16, tag=f"vaug{tt}",
                             name=f"vaug{tt}") for tt in range(TT)]

        def v_chunk(tt):
            ps = ppsum.tile([128, 512], F32, tag="proj", name="vps")
            for c in range(CT):
                nc.tensor.matmul(ps, lhsT=xsl(c, tt * 128, (tt + 1) * 128),
                                 rhs=w_sb[("wv", c)], start=(c == 0),
                                 stop=(c == CT - 1))
            nc.vector.tensor_copy(
                out=vaug[tt][:, :, :],
                in_=ps.rearrange("p (h d) -> p h d", h=HPC))

        qt_sb = [persist.tile([128, T], BF16, tag="qt", bufs=2, name=f"qt{p}")
                 for p in range(PAIRS)]
        kt_sb = [persist.tile([128, T], BF16, tag="kt", bufs=2, name=f"kt{p}")
                 for p in range(PAIRS)]

        def project_chunk(p, dst, wname, qc):
            ps = ppsum.tile([128, 512], F32, tag="proj", name="qkps")
            for c in range(CT):
                nc.tensor.matmul(
                    ps,
                    lhsT=w_sb[(wname, c)][:, p * 128:(p + 1) * 128],
                    rhs=xsl(c, qc * 512, (qc + 1) * 512),
                    start=(c == 0), stop=(c == CT - 1))
            nc.vector.tensor_copy(out=dst[:, qc * 512:(qc + 1) * 512], in_=ps)

        # PE warmup on the first wk chunk while the DMA streams in (HAM
        # unthrottle needs ~3.4us of PE busy; also bridges the DMA wait)
        warm = ppsum.tile([128, 512], F32, tag="proj", name="warm")
        for i in range(30):
            nc.tensor.matmul(warm, lhsT=w_sb[("wk", 0)][:, 0:128],
                             rhs=w_sb[("wk", 0)], start=(i == 0), stop=(i == 29))
        # pre-load the exp ACT table set during the DMA window (input is
        # uninitialized scratch; only the side-effect table load matters)
        tl = work.tile([1, 512], F32, tag="ssb", name="tabload")
        nc.scalar.activation(out=tl, in_=tl, func=EXP)

        project_chunk(0, kt_sb[0], "wk", 0)
        project_chunk(0, kt_sb[0], "wk", 1)
        project_chunk(0, qt_sb[0], "wq", 0)

        # ---- attention ----
        exps = persist.tile([128, RING * 512], BF16, tag="expS", name="expS")
        tht_sb = [persist.tile([128, T], BF16, tag=f"tht{p}", name=f"tht{p}")
                  for p in range(PAIRS)]

        fillers = {
            (0, 0): [(0, "wq", 1)],
            (0, 1): [(0, "wq", 2), (1, "wk", 0), (1, "wk", 1)],
            (0, 2): [(0, "wq", 3), (1, "wk", 2), (1, "wk", 3)],
            (0, 3): [(1, "wq", 0), (1, "wq", 1), (1, "wq", 2)],
            (1, 0): [(1, "wq", 3), (2, "wk", 0)],
            (1, 1): [(2, "wk", 1), (2, "wk", 2)],
            (1, 2): [(2, "wk", 3), (2, "wq", 0)],
            (1, 3): [(2, "wq", 1), (2, "wq", 2)],
            (2, 0): [(2, "wq", 3), (3, "wk", 0)],
            (2, 1): [(3, "wk", 1), (3, "wk", 2)],
            (2, 2): [(3, "wk", 3), (3, "wq", 0)],
            (2, 3): [(3, "wq", 1), (3, "wq", 2)],
            (3, 0): [(3, "wq", 3)],
        }

        def out_proj_group(tt):
            ysb = work.tile([128, C], F32, tag="ysb", bufs=2, name="ysb")
            for jc in range(JC):
                jsl = slice(jc * 512, (jc + 1) * 512)
                yps = ppsum.tile([128, 512], F32, tag="proj", name="yps")
                for pp in range(PAIRS):
                    nc.tensor.matmul(
                        yps, lhsT=tht_sb[pp][:, tt * 128:(tt + 1) * 128],
                        rhs=wo_sb[pp][:, jsl],
                        start=(pp == 0), stop=(pp == PAIRS - 1))
                nc.vector.tensor_add(out=ysb[:, jsl], in0=yps,
                                     in1=bias_bc[:, jsl])
            eng = nc.sync if tt % 2 == 0 else nc.gpsimd
            eng.dma_start(out=y_d[tt * 128:(tt + 1) * 128, :], in_=ysb)

        sections = [(p, qc) for p in range(PAIRS) for qc in range(QC)]

        def sec_base(si):
            return (si % 2) * 32

        def unit(si, kt, lh):
            u = sec_base(si) + 2 * kt + lh
            return exps[:, u * 512:(u + 1) * 512]

        def emit_dtree(si):
            # bf16 unit-tree, in-place forward compaction in the ring: after
            # the PV windows have read section si's 32 units, fold each
            # head's 16 units down to 2 (at unit offsets lh + {0, 16}).
            # exp of section si+2 reuses these units and naturally waits via
            # subtile deps on the M=1 chains that consume them.
            b = sec_base(si)
            uv = exps[:, b * 512:(b + 32) * 512].rearrange(
                "p (u q) -> p u q", q=512)
            with nc.allow_low_precision(reason="bf16 exp-sum tree, ~0.3% on D"):
                for lh in range(2):
                    nc.vector.tensor_add(out=uv[:, lh:lh + 29:4, :],
                                         in0=uv[:, lh:lh + 29:4, :],
                                         in1=uv[:, lh + 2:lh + 31:4, :])
                    nc.vector.tensor_add(out=uv[:, lh:lh + 25:8, :],
                                         in0=uv[:, lh:lh + 25:8, :],
                                         in1=uv[:, lh + 4:lh + 29:8, :])
                    nc.vector.tensor_add(out=uv[:, lh:lh + 17:16, :],
                                         in0=uv[:, lh:lh + 17:16, :],
                                         in1=uv[:, lh + 8:lh + 25:16, :])

        def pv_window(si, p, kt, ops):
            for lh in range(2):
                nc.tensor.matmul(
                    ops[64 * lh:64 * (lh + 1), :],
                    lhsT=vaug[kt][:, 2 * p + lh, :],
                    rhs=unit(si, kt, lh),
                    start=(kt == 0), stop=(kt == TT - 1))

        def emit_dchains(si):
            # denominators: one M=1 ones chain per head over the compacted
            # ring units (waits on emit_dtree via subtile deps)
            dps = ppsum.tile([128, 512], F32, tag="proj", name="dps")
            b = sec_base(si)
            for lh in range(2):
                for u in range(2):
                    uu = b + lh + 16 * u
                    nc.tensor.matmul(
                        dps[32 * lh:32 * lh + 1, :],
                        lhsT=ones[:, :],
                        rhs=exps[:, uu * 512:(uu + 1) * 512],
                        start=(u == 0), stop=(u == 1))
            return dps

        def finish_pvd(p, qc, ops, dps):
            qsl = slice(qc * 512, (qc + 1) * 512)
            for lh in range(2):
                ssb = work.tile([1, 512], F32, tag="ssb", name="ssb")
                nc.vector.tensor_copy(out=ssb, in_=dps[32 * lh:32 * lh + 1, :])
                rsb = work.tile([1, 512], F32, tag="rsb", name="rsb")
                nc.vector.reciprocal_approx_fast(out=rsb, in_=ssb)
                rbc = work.tile([64, 512], F32, tag="rbc", name="rbc")
                nc.gpsimd.partition_broadcast(rbc, rsb)
                nc.vector.tensor_mul(
                    out=tht_sb[p][lh * 64:(lh + 1) * 64, qsl],
                    in0=ops[64 * lh:64 * (lh + 1), :], in1=rbc)

        for si, (p, qc) in enumerate(sections):
            qsl = slice(qc * 512, (qc + 1) * 512)
            prev = sections[si - 1] if si > 0 else None
            ops = None
            if prev is not None:
                # PVD of the previous section runs at the TOP of this one:
                # its exps are fully ready, so the whole windows -> dtree ->
                # M=1 -> normalize chain completes by mid-section and never
                # gates the next section's exp (ring WAR) or PV (bank WAR)
                ops = pvsum.tile([128, 512], F32, tag="pv", name="ops")
                for kt in range(TT):
                    pv_window(si - 1, prev[0], kt, ops)
                emit_dtree(si - 1)

            for kt in range(TT):
                if p == 0 and qc == 0 and kt in (8, 12):
                    project_chunk(0, kt_sb[0], "wk", kt // 4)
                ps = spsum.tile([128, 1024], F32, tag="mm", name="sps")
                for lh in range(2):
                    hsl = slice(lh * 64, (lh + 1) * 64)
                    nc.tensor.matmul(
                        ps[:, lh * 512:(lh + 1) * 512],
                        lhsT=kt_sb[p][hsl, kt * 128:(kt + 1) * 128],
                        rhs=qt_sb[p][hsl, qsl],
                        start=True, stop=True)
                u0 = sec_base(si) + 2 * kt
                nc.scalar.activation(
                    out=exps[:, u0 * 512:(u0 + 2) * 512],
                    in_=ps, func=EXP, scale=0.125)
                if p == 0 and qc == 0 and kt >= 6:
                    v_chunk(kt - 6)
                if prev is not None and kt == 3:
                    dps = emit_dchains(si - 1)
                if prev is not None and kt == 5:
                    finish_pvd(prev[0], prev[1], ops, dps)
            if p == 0 and qc == 0:
                for tt in range(TT - 6, TT):
                    v_chunk(tt)
            # out-proj trails pair-3 sections by one: tht3[qc-1] is finalized
            # by finish_pvd at kt==5 of this section
            if p == PAIRS - 1 and qc >= 1:
                for tt in range(4 * (qc - 1), 4 * qc):
                    out_proj_group(tt)
            for fp, wname, fqc in fillers.get((p, qc), []):
                dst = qt_sb[fp] if wname == "wq" else kt_sb[fp]
                project_chunk(fp, dst, wname, fqc)

        # tail: PVD of the last section, then the final out-proj groups
        si = len(sections) - 1
        ops = pvsum.tile([128, 512], F32, tag="pv", name="ops")
        for kt in range(TT):
            pv_window(si, PAIRS - 1, kt, ops)
        emit_dtree(si)
        dps = emit_dchains(si)
        finish_pvd(PAIRS - 1, QC - 1, ops, dps)
        for tt in range(4 * (QC - 1), 4 * QC):
            out_proj_group(tt)


def _build():
    nc = bacc.Bacc("TRN2", target_bir_lowering=False)
    xw_d = nc.dram_tensor("xw", [128, CT * T], BF16, kind="ExternalInput")
    wall_d = nc.dram_tensor("wall", [128, 16384], BF16, kind="ExternalInput")
    bias_d = nc.dram_tensor("bias", [1, C], F32, kind="ExternalInput")
    y_d = nc.dram_tensor("y", [T, C], F32, kind="ExternalOutput")
    with tile.TileContext(nc) as tc:
        _emit(nc, tc, xw_d, wall_d, bias_d, y_d)
    if not nc.is_finalized():
        nc.finalize()
    return nc


def get_nc():
    global _CACHED_NC
    if _CACHED_NC is None:
        _CACHED_NC = _build()
    return _CACHED_NC


def make_in_maps(x, w_qkv, w_out, b_out):
    bf = ml_dtypes.bfloat16
    x = np.asarray(x, dtype=np.float32)
    w_qkv = np.asarray(w_qkv, dtype=np.float32)
    w_out = np.asarray(w_out, dtype=np.float32)
    b_out = np.asarray(b_out, dtype=np.float32)
    in_maps = []
    for core in range(8):
        b, hg = core // 2, core % 2
        cs = slice(hg * 512, (hg + 1) * 512)
        bias = b_out if hg == 0 else np.zeros_like(b_out)
        xT = np.ascontiguousarray(x[b].T).astype(bf)  # [C, T]
        # xw: wave-major pack xw[p, w*8192 + i*1024 + t] = xT[i*128+p, w*1024+t]
        xw = np.ascontiguousarray(
            xT.reshape(CT, 128, 2, 1024).transpose(1, 2, 0, 3)
            .reshape(128, CT * T))
        wq = w_qkv[:, 0 * C:][:, cs].astype(bf).reshape(CT, 128, 512)
        wk = w_qkv[:, 1 * C:][:, cs].astype(bf).reshape(CT, 128, 512)
        wv = w_qkv[:, 2 * C:][:, cs].astype(bf).reshape(CT, 128, 512)
        wo = w_out[cs, :].astype(bf).reshape(4, 128, C)
        wall = np.concatenate(
            [wk.transpose(1, 0, 2).reshape(128, 4096),
             wq.transpose(1, 0, 2).reshape(128, 4096),
             wv.transpose(1, 0, 2).reshape(128, 4096),
             wo.transpose(1, 0, 2).reshape(128, 4096)], axis=1)
        in_maps.append({
            "xw": xw,
            "wall": np.ascontiguousarray(wall),
            "bias": np.ascontiguousarray(bias.reshape(1, C), dtype=np.float32),
        })
    return in_maps


def _ensure_ntff_hook():
    """Register the axon NTFF profile hook if the container's antenv lacks
    axon_hooks (test/profiling use only; never needed for plain kernel())."""
    import sys
    import types
    try:
        from antenv import axon_hooks  # noqa: F401
    except ImportError:
        mod = types.ModuleType("antenv.axon_hooks")
        mod._hook = None

        def set_axon_ntff_profile_hook(hook, _m=mod):
            _m._hook = hook

        def get_axon_ntff_profile_hook(_m=mod):
            return _m._hook

        mod.set_axon_ntff_profile_hook = set_axon_ntff_profile_hook
        mod.get_axon_ntff_profile_hook = get_axon_ntff_profile_hook
        sys.modules["antenv.axon_hooks"] = mod
        import antenv
        antenv.axon_hooks = mod
    import antenv.axon_hooks as ah
    if ah.get_axon_ntff_profile_hook() is None:
        from trn_agent_boot.trn_boot import _ntff_profile_via_ctypes
        ah.set_axon_ntff_profile_hook(
            _ntff_profile_via_ctypes("/opt/axon/libaxon_pjrt.so"))


def kernel(x, w_qkv, w_out, b_out, _trace=False, _trace_kwargs=None):
    nc = get_nc()
    in_maps = make_in_maps(x, w_qkv, w_out, b_out)
    kwargs = {}
    if _trace:
        try:
            _ensure_ntff_hook()
        except Exception as e:
            print(f"NTFF hook setup failed ({e}); running without trace")
        else:
            kwargs.update(trace=True, **(_trace_kwargs or {}))
    res = run_bass_kernel_spmd(nc, in_maps, core_ids=list(range(8)), **kwargs)
    out = np.empty((B, T, C), dtype=np.float32)
    for b in range(B):
        out[b] = res.results[2 * b]["y"] + res.results[2 * b + 1]["y"]
    if _trace:
        return out, res
    return out
